# revision 20
# baseline (speedup 1.0000x reference)
"""Masked dot-product attention (B=16, Lq=Lk=2048, d=64) on 8 TRN2 NeuronCores.

Distribution
------------
Attention rows are independent, so work is split into 64 units = (batch,
512-query chunk). Unit cost = ceil(valid_len/128) k-tiles; fully-masked
k-tiles contribute exactly zero and are skipped. Units are sorted by cost
(ascending) and snake-assigned to 8 slots x 8 cores; each slot's tile
count is the max within the slot, so all 8 cores run ONE shared SPMD
program (per-core differences live only in the staged data).

Device math per unit (S^T formulation; softmax over the partition axis):
    s_t[k, q]  = (K^T_t weights) @ Q^T           (PE, fp32r, PSUM)
    p_t[k, q]  = exp(0.125 * s_t)                (ACT, PSUM->SBUF)
    pv[v, q]  += V'_t^T @ p_t                    (PE, accumulate over t)
where V'_t = [V rows | ones], with rows >= valid_len zeroed on the host —
this applies the key mask AND computes the softmax denominator l = pv[64]
inside the same matmul. No row-max subtraction is needed: scores are
O(+-10) (exact softmax shift-invariance; masked lanes match the
reference's exp(-1e6)->0). valid_len == 0 reproduces jax's uniform
softmax by zeroing Q (s = 0 -> p = 1) and leaving V' unmasked.

Performance notes (measured on this axon-tunneled TRN2):
- The S matmuls are K=64 (d=64 contraction) and only occupy half the
  128-row PE array, so consecutive tiles are ROW-TILED: odd GLOBAL tiles
  carry K^T at SBUF partitions 64-127 (host staging) and Q^T is staged
  doubled into both partition halves; the two matmuls of a pair run
  concurrently in disjoint array halves (~336ns/pair vs 854ns serial).
- The PV matmul is split the same way: pv_lo += V'[0:64]^T @ p[0:64] and
  pv_hi += V'[64:128]^T @ p[64:128] are two K=64 matmuls in disjoint
  array halves accumulating into SEPARATE PSUM banks; they co-stream
  like the S pairs (~336ns/pair vs ~427ns for the single K=128 matmul).
  The two banks are summed by one DVE tensor_add per slot.
- NO on-device normalization: each slot ships the raw [65, 512] numerator
  ‖ denominator block to DRAM and the host does out = (pv[0:64]/pv[64]).T
  in numpy. This removes the reciprocal/broadcast/DRAM-roundtrip epilogue
  chain that serialized the tail of the old kernel (~12us of PE idle).
- PSUM budget: s groups of 2 tiles ([128,1024] = 2 banks) x 2 bufs +
  pv_lo/pv_hi ([128,512] = 1 bank) x 2 bufs each = 8 banks exactly.
  GROUP=2 is forced by this budget; the ACT engine is the steady-state
  pacer at ~427ns/tile (1 col/cycle @1.2GHz, fp32 PSUM reads) + ~185ns
  per-instruction access latency, so the pipeline is ACT-bound.
- All staged operands are bf16 (fp32 staging measured DMA/LDW-heavier
  with identical PE stream rate; rel-err ~3e-3 vs the 2e-2 gate).
- kv k-tiles are staged host-side as [V'|K^T] combined 386B partition
  rows, DMA'd in 2-tile chunks (one per group; a single DMA stream runs
  at only ~63GB/s, so 3-tile chunks could not keep up with the group
  cadence); chunk DMAs alternate between the Sync and GpSimd queues.
  Q^T tiles come from a 3-deep rotating pool (a single resident Q tile
  serialized every prefetch behind ALL in-flight S reads). Output DMAs
  are column-split across Sync/GpSimd and their issue is DEFERRED ~5
  group-iterations so a not-yet-satisfied wait never blocks the kv
  chunks behind it in the in-order queue; the final slot's outs ride
  Sync only, keeping GpSimd's ~3us software-DGE drain off the tail.
  Slot 0's Q^T halves ride Scalar+Sync ahead of the exp-table warm-up
  to cut the post-preamble fill (the framework preamble itself — engine
  rendezvous barriers + register loads — costs a fixed ~7us).
- The PE instruction queue is in-order, so PV matmuls of group g are
  emitted after the S matmuls of group g+2 (two-group software pipeline);
  the PE never stalls on the exp.
- Every TPB instruction may carry at most ONE sync wait on this walrus;
  split_multi_waits() post-processes the scheduled program into
  single-wait form with wait-carrying NoOps.
"""
import ml_dtypes
import numpy as np

import concourse.bass as bass
import concourse.mybir as mybir
import concourse.tile as tile
from concourse.bass_utils import run_bass_kernel_spmd


def split_multi_waits(nc):
    """TRN2 TPB instructions encode a single sync-wait slot. Tile's
    add_semaphores can emit several waits on one instruction (and the
    kernel-tail drain aggregates one per live proc), which walrus rejects
    ("Too many sync wait commands"). Rewrite every instruction carrying
    k>1 waits into (k-1) same-engine NoOps carrying one wait each."""
    for fn in nc.m.functions:
        for bb in fn.blocks:
            new = []
            for inst in bb.instructions:
                si = inst.sync_info
                ow = list(si.on_wait) if si else []
                if len(ow) > 1:
                    for jj, w in enumerate(ow[:-1]):
                        nop = mybir.InstNoOp(
                            name=f"{inst.name}_sw{jj}", ins=[], outs=[])
                        nop.engine = inst.engine
                        nop.sync_info = mybir.SyncInfo(
                            on_wait=[w], on_update=[])
                        new.append(nop)
                    inst.sync_info = mybir.SyncInfo(
                        on_wait=[ow[-1]], on_update=list(si.on_update))
                new.append(inst)
            bb.instructions = new

F32 = mybir.dt.float32
F32R = mybir.dt.float32r
BF16 = mybir.dt.bfloat16

B, L, D = 16, 2048, 64
QC = 512                 # query-chunk (free dim of both matmuls)
NQCHUNK = L // QC        # 4 chunks per batch
KT = 128                 # k rows per tile
N_CORES = 8
N_SLOTS = (B * NQCHUNK) // N_CORES   # 8 units per core
GROUP = 2                # k-tiles per ACT/PSUM group (2x2 s banks)
CHUNK = 2                # k-tiles per kv DMA (single-stream DMA is ~63GB/s;
                         # 2-tile chunks keep per-queue transfer < group period)
KV_W = 65 + 128          # combined staged tile width: [V'|K^T]


def _schedule(valid_lens):
    """Snake-assign 64 units to 8 slots x 8 cores. Returns (N_list, assign)
    where assign[core][slot] = (batch, qchunk) and N_list[slot] = tile
    count every core runs for that slot."""
    evl = np.where(valid_lens > 0, valid_lens, L).astype(np.int64)
    cost = np.ceil(evl / KT).astype(np.int64)        # per batch
    units = [(int(cost[b]), b, qc) for b in range(B) for qc in range(NQCHUNK)]
    units.sort(key=lambda t: (t[0], t[1], t[2]))
    N_list = []
    assign = [[None] * N_SLOTS for _ in range(N_CORES)]
    for j in range(N_SLOTS):
        grp = units[j * N_CORES:(j + 1) * N_CORES]
        N_list.append(grp[-1][0])
        for c in range(N_CORES):
            _, b, qc = grp[c]
            assign[c][j] = (b, qc)
    return N_list, assign


_PROGRAM_CACHE = {}


def _build_program(N_list):
    key = tuple(N_list)
    if key in _PROGRAM_CACHE:
        return _PROGRAM_CACHE[key]
    # kv staged chunk-major, CHUNK tiles contiguous within a partition row
    # so one DMA moves up to 2.3KB/partition; chunks never span slots
    n_chunks = [int(np.ceil(n / CHUNK)) for n in N_list]
    TC = int(sum(n_chunks))
    coff = [0]
    for g in n_chunks:
        coff.append(coff[-1] + g)
    toff = [0]
    for n in N_list:
        toff.append(toff[-1] + n)

    nc = bass.Bass()
    kv_d = nc.declare_dram_parameter("kv", [TC, KT, CHUNK * KV_W], BF16,
                                     isOutput=False)
    qT_d = nc.declare_dram_parameter("qT", [N_SLOTS, KT, QC], BF16,
                                     isOutput=False)
    pv_d = nc.declare_dram_parameter("pv", [N_SLOTS, 65, QC], F32,
                                     isOutput=True)

    with tile.TileContext(nc) as tc:
        with (
            tc.tile_pool(name="kv_pool", bufs=12) as kv_pool,
            tc.tile_pool(name="q_pool", bufs=3) as q_pool,
            tc.tile_pool(name="p_pool", bufs=5) as p_pool,
            tc.tile_pool(name="ep_pool", bufs=2) as ep_pool,
            tc.tile_pool(name="warm_pool", bufs=1) as warm_pool,
            tc.tile_pool(name="s_pool", bufs=2, space="PSUM") as s_pool,
            tc.tile_pool(name="pv_pool", bufs=2, space="PSUM") as pv_pool,
        ):

            # Per-slot Q^T tiles from a rotating pool: a fresh tile per
            # slot means the prefetch DMA has NO dependency on the previous
            # slot's matmul reads (a single resident tile serialized every
            # qT DMA behind ALL in-flight S reads, stalling the kv chunk
            # DMAs queued behind it for ~5us per run).
            qts = [None] * N_SLOTS
            qts[0] = q_pool.tile([KT, QC], BF16, tag="qt", name="qt0")
            # split across the Scalar+Sync hardware-DGE queues: single-
            # stream DMA is ~63GB/s, halving the first transfer halves the
            # startup stall; Scalar's DMA is emitted BEFORE the exp-table
            # warm-up so it issues the moment the preamble ends
            nc.scalar.dma_start(
                out=qts[0][0:D, :],
                in_=bass.AP(tensor=qT_d, offset=0, ap=[[QC, D], [1, QC]]))
            nc.sync.dma_start(
                out=qts[0][D:, :],
                in_=bass.AP(tensor=qT_d, offset=D * QC,
                            ap=[[QC, D], [1, QC]]))

            # ACT exp-table warm-up: overlap the one-time table load with
            # the first DMAs instead of stalling the first real group.
            warm = warm_pool.tile([1, 1], F32)
            nc.vector.memset(warm, 0.0)
            nc.scalar.activation(warm, warm, mybir.ActivationFunctionType.Exp)


            # two-group-deep software pipeline: PV matmuls of group g are
            # emitted after the S matmuls of group g+2, so the in-order PE
            # queue never waits on the exp.
            PIPE_DEPTH = 2
            pending = []       # [(pv_lo, pv_hi, kvs, p, t0, n, g, j), ...]
            # first kv chunk rides GpSimd so it overlaps slot 0's Q^T
            # load on Sync; first S matmul is gated on max, not sum
            dma_flip = [1]

            def kv_dma(out, in_):
                eng = nc.sync if dma_flip[0] % 2 == 0 else nc.gpsimd
                dma_flip[0] += 1
                eng.dma_start(out=out, in_=in_)

            def flush_one():
                if not pending:
                    return
                pv_lo, pv_hi, kvs, p, t0, n, g, _ = pending.pop(0)
                for i in range(g):
                    nc.tensor.matmul(pv_lo[0:65, :],
                                     lhsT=kvs[i][0:D, 0:65],
                                     rhs=p[0:D, i * QC:(i + 1) * QC],
                                     start=(t0 + i == 0),
                                     stop=(t0 + i == n - 1))
                    nc.tensor.matmul(pv_hi[0:65, :],
                                     lhsT=kvs[i][D:, 0:65],
                                     rhs=p[D:, i * QC:(i + 1) * QC],
                                     start=(t0 + i == 0),
                                     stop=(t0 + i == n - 1))

            epilogues = []     # (j, pv_lo, pv_hi) awaiting PV flush
            out_defer = []     # (ready_iter, j, pvc, half) pending out DMAs
            it = [0]           # group-iteration counter
            HC = QC // 2

            def flush_outs(final=False):
                # out DMAs are emitted >= 2 group-iterations after their
                # epilogue so the issue's wait is already satisfied — an
                # out DMA stuck waiting on the DVE add would block every
                # kv chunk queued behind it on the same in-order queue
                while out_defer and (final or it[0] >= out_defer[0][0]):
                    _, j, pvc, c0, c1, eng_i = out_defer.pop(0)
                    eng = (nc.gpsimd, nc.sync, nc.scalar)[eng_i]
                    eng.dma_start(out=pv_d[j][:, c0:c1], in_=pvc[:, c0:c1])

            def emit_epilogues():
                # a slot's epilogue may only run once every PV group of its
                # unit has been flushed (program order defines semantics);
                # it frees both PSUM accumulator banks. Column-split into
                # halves: two ~340ns DVE ops per half, and the two out DMAs
                # ride different queues (~1.05us each at the ~63GB/s
                # single-stream rate instead of 2.1us serial).
                while epilogues and (not pending
                                     or epilogues[0][0] < pending[0][7]):
                    j, pv_lo, pv_hi = epilogues.pop(0)
                    # DVE may read only ONE operand from PSUM per
                    # instruction (NCC_IBVF027): copy lo, then add hi.
                    pieces = 2
                    pw = QC // pieces
                    pvc = ep_pool.tile([65, QC], F32, tag="pvc")
                    for h in range(pieces):
                        cs = slice(h * pw, (h + 1) * pw)
                        nc.vector.tensor_copy(pvc[:, cs], pv_lo[0:65, cs])
                        nc.vector.tensor_add(pvc[:, cs], pvc[:, cs],
                                             pv_hi[0:65, cs])
                        # +5: the DVE add EXECUTES ~4 group-iterations
                        # after this point is EMITTED (2-deep PV pipeline +
                        # DVE queue lag); a DMA issued earlier would sit
                        # blocked at the head of its queue
                        # final slot: Scalar+Sync (both hardware-DGE, and
                        # ACT is idle by the tail) so the two last transfers
                        # run in PARALLEL queues. GpSimd must never own the
                        # program's last DMA — its software-DGE drain costs
                        # ~2-3us after its final transfer.
                        if j == N_SLOTS - 1:
                            eng_i = 2 if h == 0 else 1
                        else:
                            eng_i = 1 if h % 2 == 1 else 0
                        out_defer.append((it[0] + 5, j, pvc, h * pw,
                                          (h + 1) * pw, eng_i))

            for j in range(N_SLOTS):
                if j + 1 < N_SLOTS:
                    # prefetch next slot's Q^T (one small DMA per slot;
                    # kv chunk DMAs alternate between Sync and GpSimd)
                    qts[j + 1] = q_pool.tile([KT, QC], BF16, tag="qt",
                                             name=f"qt{j + 1}")
                    nc.sync.dma_start(
                        out=qts[j + 1],
                        in_=bass.AP(tensor=qT_d, offset=(j + 1) * KT * QC,
                                    ap=[[QC, KT], [1, QC]]))
                n = N_list[j]
                pv_lo = pv_pool.tile([KT, QC], F32, tag="pv_lo")
                pv_hi = pv_pool.tile([KT, QC], F32, tag="pv_hi")
                kvg = None
                t = 0
                while t < n:
                    g = min(GROUP, n - t)
                    kvs = []
                    for i in range(g):
                        ci = (t + i) // CHUNK
                        cr = (t + i) % CHUNK
                        if cr == 0:
                            kvg = kv_pool.tile([KT, CHUNK * KV_W], BF16,
                                               tag="kv")
                            w = min(CHUNK, n - (t + i)) * KV_W
                            if j == 0 and t + i == 0 and w > KV_W:
                                # startup: split the first chunk across
                                # both queues so the first S matmul waits
                                # on a ~0.8us transfer, not a ~1.5us one
                                nc.gpsimd.dma_start(
                                    out=kvg[:, 0:KV_W],
                                    in_=kv_d[0][:, 0:KV_W])
                                nc.sync.dma_start(
                                    out=kvg[:, KV_W:w],
                                    in_=kv_d[0][:, KV_W:w])
                            else:
                                kv_dma(kvg[:, 0:w],
                                       kv_d[coff[j] + ci][:, 0:w])
                        kvs.append(kvg[:, cr * KV_W:(cr + 1) * KV_W])
                    flush_outs()
                    s = s_pool.tile([KT, GROUP * QC], F32, tag="s")
                    for i in range(g):
                        # odd GLOBAL tiles carry K^T at partitions 64-127
                        # (staged by the host) so consecutive S matmuls
                        # occupy disjoint PE row halves and overlap in the
                        # array; global parity keeps the lo/hi alternation
                        # unbroken across slot boundaries
                        lo = D * ((toff[j] + t + i) % 2)
                        nc.tensor.matmul(s[:, i * QC:(i + 1) * QC],
                                         lhsT=kvs[i][lo:lo + D, 65:],
                                         rhs=qts[j][lo:lo + D, :],
                                         start=True, stop=True)
                    p = p_pool.tile([KT, GROUP * QC], BF16, tag="p")
                    nc.scalar.activation(p[:, 0:g * QC], s[:, 0:g * QC],
                                         mybir.ActivationFunctionType.Exp,
                                         scale=0.125)
                    if len(pending) >= PIPE_DEPTH:
                        flush_one()
                        emit_epilogues()
                    pending.append((pv_lo, pv_hi, kvs, p, t, n, g, j))
                    t += g
                    it[0] += 1
                epilogues.append((j, pv_lo, pv_hi))
            while pending:
                flush_one()
                emit_epilogues()
            emit_epilogues()
            flush_outs(final=True)

    split_multi_waits(nc)
    _PROGRAM_CACHE[key] = (nc, coff, toff)
    return nc, coff, toff


def _stage_inputs(queries, keys, values, valid_lens, N_list, assign, coff,
                  toff):
    evl = np.where(valid_lens > 0, valid_lens, L).astype(np.int64)
    zero_q = valid_lens <= 0
    TC = coff[-1]

    # Per-batch precomputed host tensors
    kTT = np.ascontiguousarray(keys.transpose(0, 2, 1))        # [B, D, L]
    vmask = (np.arange(L)[None, :] < evl[:, None])             # [B, L]
    vp = np.concatenate(
        [values, np.ones((B, L, 1), np.float32)], axis=2)      # [B, L, 65]
    vp = vp * vmask[:, :, None].astype(np.float32)

    in_maps = []
    for c in range(N_CORES):
        # chunk-major: kv[ch][p][i*KV_W:(i+1)*KV_W] = tile (3ch+i): [V'|K^T]
        kv = np.zeros((TC, KT, CHUNK, KV_W), ml_dtypes.bfloat16)
        qT = np.zeros((N_SLOTS, KT, QC), ml_dtypes.bfloat16)
        for j in range(N_SLOTS):
            b, qc = assign[c][j]
            n_real = int(np.ceil(evl[b] / KT))
            if not zero_q[b]:
                qT[j, 0:D] = queries[b, qc * QC:(qc + 1) * QC, :].T
                qT[j, D:] = qT[j, 0:D]
            n = min(n_real, N_list[j])
            ncap = CHUNK * (coff[j + 1] - coff[j])
            vt = np.zeros((ncap, KT, 65), np.float32)
            kt = np.zeros((ncap, D, KT), np.float32)
            vt[0:n] = vp[b, 0:n * KT].reshape(n, KT, 65)
            kt[0:n] = kTT[b, :, 0:n * KT].reshape(D, n, KT).transpose(1, 0, 2)
            sl = slice(coff[j], coff[j + 1])
            ng = coff[j + 1] - coff[j]
            kv[sl, :, :, 0:65] = vt.reshape(ng, CHUNK, KT, 65).transpose(0, 2, 1, 3)
            ktg = kt.reshape(ng, CHUNK, D, KT).transpose(0, 2, 1, 3)
            # K^T half placement follows GLOBAL tile parity (toff[j] + t)
            par = (toff[j] + np.arange(ncap)) % 2
            for i in range(CHUNK):
                pi = par.reshape(ng, CHUNK)[:, i]
                lo_rows = pi == 0
                kv[sl, 0:D, i, 65:][lo_rows] = ktg[:, :, i, :][lo_rows]
                kv[sl, D:, i, 65:][~lo_rows] = ktg[:, :, i, :][~lo_rows]
        in_maps.append({"kv": kv.reshape(TC, KT, CHUNK * KV_W), "qT": qT})
    return in_maps


def _gather(results, assign):
    out = np.empty((B, L, D), np.float32)
    for c in range(N_CORES):
        pv = results[c]["pv"]                     # [N_SLOTS, 65, QC]
        for j in range(N_SLOTS):
            b, qc = assign[c][j]
            blk = pv[j]
            out[b, qc * QC:(qc + 1) * QC, :] = (blk[0:64] / blk[64:65]).T
    return out


def run(queries, keys, values, valid_lens, trace=False):
    queries = np.asarray(queries, np.float32)
    keys = np.asarray(keys, np.float32)
    values = np.asarray(values, np.float32)
    valid_lens = np.asarray(valid_lens)
    N_list, assign = _schedule(valid_lens)
    nc, coff, toff = _build_program(N_list)
    in_maps = _stage_inputs(queries, keys, values, valid_lens, N_list,
                            assign, coff, toff)
    res = run_bass_kernel_spmd(nc, in_maps, list(range(N_CORES)),
                               trace=trace)
    return _gather(res.results, assign), res


def kernel(queries, keys, values, valid_lens):
    out, _ = run(queries, keys, values, valid_lens)
    return out
